# revision 9
# baseline (speedup 1.0000x reference)
"""CSWin self-attention Trainium2 kernel.

Sharding: data-parallel over batch B=8 across 8 cores (1 image per core).
Per-core pipeline (image = 128x128 spatial, C=256):
  A) LayerNorm (gamma folded into Wqkv on host): bn_stats/aggr (DVE),
     batched rstd (one Sqrt + recip per 8 tiles), normalize on GPSIMD,
     PE-transpose, then two evictions of y^T: row-major yt (for the
     horizontal pass) and column-major ytT (for the vertical pass, so
     its qkv matmuls stream contiguously).
  B) Vertical stripes then horizontal stripes (+fused projection),
     processed as 32 "superstripes" of 2 stripes each:
       q^T/k^T matmuls at N=512 across both stripes, v token-major,
       one bf16 CAST for q/k + one for v,
       per stripe: S^T row-tiled 4 heads (K=32), exp on ScalarE
       ([128,2048] ACTIVATE, scale folded),
       attn@V col-tiled 4 heads + replicated denominators (ones lhsT)
       into a shared psum tile, batched reciprocal_approx_fast +
       tensor_mul straight from psum.
  C) (fused into horizontal pass) projection h @ Wproj + residual via
     float32r identity matmul on re-read x (full-speed at N=256);
     psum banks reused v->attn out->proj; output staged and DMA'd per
     superstripe.
"""

import math
from contextlib import ExitStack

import numpy as np
import ml_dtypes

import concourse.bass as bass
import concourse.bacc as bacc
import concourse.mybir as mybir
import concourse.tile as tile
from concourse.bass_utils import run_bass_kernel_spmd

F32 = mybir.dt.float32
F32R = mybir.dt.float32r
BF16 = mybir.dt.bfloat16
AF = mybir.ActivationFunctionType
ALU = mybir.AluOpType

B = 8
HH = 128
WW = 128
C = 256
T = HH * WW         # 16384 tokens
NT = T // 128       # 128 token tiles
NS = 64             # stripes per direction
NSP = NS // 2       # superstripes (2 stripes each)
SEQ = 256           # stripe seq len (2 * 128)
NHD = 4             # heads per direction
HD = 32
SCALE = HD ** -0.5
EPS = 1e-5
ABLK = 8            # tiles per rstd batch in phase A


def build_nc(has_qbias: bool, has_pbias: bool) -> bass.Bass:
    nc = bacc.Bacc("TRN2", target_bir_lowering=False, debug=False)
    x_h = nc.dram_tensor("x", [T, C], F32, kind="ExternalInput")
    xr_h = nc.dram_tensor("xr", [T, C], F32R, kind="ExternalInput")
    wqkv_h = nc.dram_tensor("wqkv", [2, 128, 768], BF16, kind="ExternalInput")
    wproj_h = nc.dram_tensor("wproj", [2, 128, 256], BF16, kind="ExternalInput")
    bqkv_h = nc.dram_tensor("bqkv", [1, 768], BF16, kind="ExternalInput")
    bproj_h = nc.dram_tensor("bproj", [1, 256], BF16, kind="ExternalInput")
    ident_h = nc.dram_tensor("ident", [128, 128], BF16, kind="ExternalInput")
    identr_h = nc.dram_tensor("identr", [128, 128], F32R, kind="ExternalInput")
    out_h = nc.dram_tensor("out", [T, C], F32, kind="ExternalOutput")

    with tile.TileContext(nc) as tc, tc.tile_pool(name="persist", bufs=1) as pp:
        # ---------------- persistent SBUF ----------------
        yt = pp.tile([128, 2, T], BF16, name="yt", tag="yt")
        ytT = pp.tile([128, 2, T], BF16, name="ytT", tag="ytT")
        hVt = pp.tile([128, T], BF16, name="hVt", tag="hVt")
        wqkv = pp.tile([128, 2 * 768], BF16, name="wqkv", tag="wqkv")
        wproj = pp.tile([128, 2 * 256], BF16, name="wproj", tag="wproj")
        brow = pp.tile([1, 768], BF16, name="brow", tag="brow")
        bprow = pp.tile([1, 256], BF16, name="bprow", tag="bprow")
        ones = pp.tile([1, 512], BF16, name="ones", tag="ones")
        ones32 = pp.tile([128, 32], BF16, name="ones32", tag="ones32")
        ident = pp.tile([128, 128], BF16, name="ident", tag="ident")
        identr = pp.tile([128, 128], F32R, name="identr", tag="identr")
        mvs = pp.tile([128, NT, 2], F32, name="mvs", tag="mvs")
        srt = pp.tile([128, NT], F32, name="srt", tag="srt")
        rstds = pp.tile([128, NT], F32, name="rstds", tag="rstds")
        epsc = pp.tile([128, 1], F32, name="epsc", tag="epsc")

        nc.sync.dma_start(out=wqkv[:, 0:768], in_=wqkv_h[0])
        nc.sync.dma_start(out=wqkv[:, 768:1536], in_=wqkv_h[1])
        nc.sync.dma_start(out=wproj[:, 0:256], in_=wproj_h[0])
        nc.sync.dma_start(out=wproj[:, 256:512], in_=wproj_h[1])
        if has_qbias:
            nc.sync.dma_start(out=brow[:], in_=bqkv_h[:])
        if has_pbias:
            nc.sync.dma_start(out=bprow[:], in_=bproj_h[:])
        nc.vector.memset(ones[:], 1.0)
        nc.vector.memset(ones32[:], 1.0)
        nc.vector.memset(epsc[:], EPS)
        nc.sync.dma_start(out=ident[:], in_=ident_h[:, :])
        nc.sync.dma_start(out=identr[:], in_=identr_h[:, :])

        ytTv = ytT[:].rearrange("p c (w h) -> p c w h", w=WW)
        x_blk = x_h[:, :].rearrange("(b t p) c -> b p t c", t=ABLK, p=128)

        # ---------------- phase A: LN + transpose ----------------
        with (
            tc.tile_pool(name="xa", bufs=2) as xa_pool,
            tc.tile_pool(name="ya", bufs=3) as ya_pool,
            tc.tile_pool(name="sa", bufs=4) as sa_pool,
            tc.tile_pool(name="tp", bufs=3, space="PSUM") as tp_pool,
        ):
            for blk in range(NT // ABLK):
                xs = xa_pool.tile([128, ABLK, 256], F32, tag="xs")
                nc.sync.dma_start(out=xs[:], in_=x_blk[blk])
                for j in range(ABLK):
                    i = blk * ABLK + j
                    st6 = sa_pool.tile([128, 6], F32, tag="st6")
                    nc.vector.bn_stats(st6[:], xs[:, j, :])
                    nc.vector.bn_aggr(mvs[:, i, :], st6[:])
                sl = slice(blk * ABLK, (blk + 1) * ABLK)
                nc.scalar.activation(srt[:, sl], mvs[:, sl, 1], AF.Sqrt,
                                     bias=epsc[:])
                nc.vector.reciprocal(rstds[:, sl], srt[:, sl])
                for j in range(ABLK):
                    i = blk * ABLK + j
                    yn = ya_pool.tile([128, 256], BF16, tag="yn")
                    nc.gpsimd.tensor_scalar(
                        yn[:], xs[:, j, :], mvs[:, i, 0:1], rstds[:, i:i + 1],
                        ALU.subtract, ALU.mult,
                    )
                    tp = tp_pool.tile([128, 2, 128], BF16, tag="tp")
                    nc.tensor.transpose(tp[:, 0, :], yn[:, 0:128], ident[:])
                    nc.tensor.transpose(tp[:, 1, :], yn[:, 128:256], ident[:])
                    nc.scalar.copy(yt[:, :, i * 128:(i + 1) * 128], tp[:])
                    nc.scalar.copy(ytTv[:, :, :, i], tp[:])

        # stripe-sliced channel-major views of y^T
        hVv = hVt[:].rearrange("p (g j h) -> p g j h", g=NS, j=2)
        out_sp = out_h[:, :].rearrange("(t j p) c -> t p j c", j=4, p=128)
        xr_sp = xr_h[:, :].rearrange("(t j p) c -> t p j c", j=4, p=128)

        def super_stripe(sp, qoff, ysrc, pools, mdst, proj_dst=None):
            """Attention for superstripe sp (stripes 2sp, 2sp+1).
            mdst: [128, 512] AP for normalized h^T of both stripes.
            proj_dst: out_sp[sp]-style AP to enable fused projection."""
            (qk_pool, vod_pool, s_pool, qksb_pool, esb_pool, dr_pool,
             ost_pool, xr_pool) = pools
            tok0 = sp * 512
            yv = ysrc[:]  # [128, 2, T]
            qk_ps = qk_pool.tile([128, 1024], F32, tag="qkps")
            vod = vod_pool.tile([128, 1024], F32, tag="vod")
            for kc in range(2):
                wof = kc * 768
                rv = yv[:, kc, tok0:tok0 + 512]
                nc.tensor.matmul(
                    qk_ps[:, 0:512], lhsT=wqkv[:, wof + qoff:wof + qoff + 128],
                    rhs=rv, start=kc == 0, stop=kc == 1 and not has_qbias)
                nc.tensor.matmul(
                    qk_ps[:, 512:1024],
                    lhsT=wqkv[:, wof + 256 + qoff:wof + 384 + qoff],
                    rhs=rv, start=kc == 0, stop=kc == 1 and not has_qbias)
                for q in range(4):  # (stripe s, seq half sc) quarters
                    nc.tensor.matmul(
                        vod[:, q * 128:q * 128 + 128],
                        lhsT=yv[:, kc, tok0 + q * 128:tok0 + (q + 1) * 128],
                        rhs=wqkv[:, wof + 512 + qoff:wof + 640 + qoff],
                        start=kc == 0, stop=kc == 1 and not has_qbias)
            if has_qbias:
                nc.tensor.matmul(
                    qk_ps[:, 0:512], lhsT=brow[:, qoff:qoff + 128],
                    rhs=ones[:, 0:512], start=False, stop=True)
                nc.tensor.matmul(
                    qk_ps[:, 512:1024], lhsT=brow[:, 256 + qoff:384 + qoff],
                    rhs=ones[:, 0:512], start=False, stop=True)
                for q in range(4):
                    nc.tensor.matmul(
                        vod[:, q * 128:q * 128 + 128],
                        lhsT=ones[:, 0:128],
                        rhs=brow[:, 512 + qoff:640 + qoff],
                        start=False, stop=True)
            qkv_sb = qksb_pool.tile([128, 1536], BF16, tag="qkvsb")
            nc.vector.tensor_copy(qkv_sb[:, 0:1024], qk_ps[:])
            nc.vector.tensor_copy(qkv_sb[:, 1024:1536], vod[:, 0:512])
            # per stripe: S^T, exp; then attn@V + denominators
            e_sbs = []
            for s in range(2):
                s_ps = s_pool.tile([128, 2048], F32, tag="sps")
                for h in range(NHD):
                    for sc in range(2):
                        nc.tensor.matmul(
                            s_ps[:, h * 512 + sc * 256:h * 512 + sc * 256 + 256],
                            lhsT=qkv_sb[32 * h:32 * h + 32,
                                        512 + s * 256 + sc * 128:
                                        512 + s * 256 + sc * 128 + 128],
                            rhs=qkv_sb[32 * h:32 * h + 32,
                                       s * 256:s * 256 + 256],
                            start=True, stop=True,
                            tile_position=(32 * h, 0))
                e_sb = esb_pool.tile([128, 2048], BF16, tag="esb")
                nc.scalar.activation(e_sb[:], s_ps[:], AF.Exp, scale=SCALE)
                e_sbs.append(e_sb)
            for s in range(2):
                e_sb = e_sbs[s]
                for sc in range(2):
                    for h in range(NHD):
                        nc.tensor.matmul(
                            vod[32 * h:32 * h + 32, s * 256:s * 256 + 256],
                            lhsT=qkv_sb[:, 1024 + s * 256 + sc * 128 + 32 * h:
                                        1024 + s * 256 + sc * 128 + 32 * h + 32],
                            rhs=e_sb[:, h * 512 + sc * 256:
                                     h * 512 + sc * 256 + 256],
                            start=sc == 0, stop=sc == 1,
                            tile_position=(0, 32 * h))
                    for h in range(NHD):
                        nc.tensor.matmul(
                            vod[32 * h:32 * h + 32,
                                512 + s * 256:512 + s * 256 + 256],
                            lhsT=ones32[:],
                            rhs=e_sb[:, h * 512 + sc * 256:
                                     h * 512 + sc * 256 + 256],
                            start=sc == 0, stop=sc == 1,
                            tile_position=(0, 32 * h))
            drec = dr_pool.tile([128, 512], F32, tag="drec")
            nc.vector.reciprocal_approx_fast(drec[:], vod[:, 512:1024])
            nc.vector.tensor_mul(mdst, vod[:, 0:512], drec[:])
            if proj_dst is None:
                return
            # ---- fused projection + residual for 4 token tiles ----
            xrs = xr_pool.tile([128, 4, 256], F32R, tag="xrs")
            nc.sync.dma_start(out=xrs[:], in_=xr_sp[sp])
            for jj in range(4):
                t = 4 * sp + jj
                po = slice(jj * 256, jj * 256 + 256)
                nc.tensor.matmul(
                    vod[:, po], lhsT=mdst[:, jj * 128:jj * 128 + 128],
                    rhs=wproj[:, 0:256], start=True, stop=False)
                nc.tensor.matmul(
                    vod[:, po], lhsT=hVv[:, :, :, t],
                    rhs=wproj[:, 256:512], start=False, stop=False)
                nc.tensor.matmul(
                    vod[:, po], lhsT=identr[:],
                    rhs=xrs[:, jj, :], start=False, stop=not has_pbias)
                if has_pbias:
                    nc.tensor.matmul(
                        vod[:, po], lhsT=ones[:, 0:128], rhs=bprow[:],
                        start=False, stop=True)
            ost = ost_pool.tile([128, 4, 256], F32, tag="ost")
            nc.scalar.copy(ost[:, 0:2, :], vod[:, 0:512])
            nc.vector.tensor_copy(ost[:, 2:4, :], vod[:, 512:1024])
            nc.sync.dma_start(out=proj_dst, in_=ost[:])

        # ---------------- phase Bv: vertical attention ----------------
        with (
            tc.tile_pool(name="qkps", bufs=1, space="PSUM") as qk_pool,
            tc.tile_pool(name="vodps", bufs=1, space="PSUM") as vod_pool,
            tc.tile_pool(name="sps", bufs=1, space="PSUM") as s_pool,
            tc.tile_pool(name="qksb", bufs=2) as qksb_pool,
            tc.tile_pool(name="esb", bufs=2) as esb_pool,
            tc.tile_pool(name="drsb", bufs=2) as dr_pool,
        ):
            pools = (qk_pool, vod_pool, s_pool, qksb_pool, esb_pool, dr_pool,
                     None, None)
            for sp in range(NSP):
                super_stripe(sp, 128, ytT, pools,
                             hVt[:, sp * 512:(sp + 1) * 512])

        # ------------- phase BhC: horizontal attention + proj -------------
        with (
            tc.tile_pool(name="qkps", bufs=1, space="PSUM") as qk_pool,
            tc.tile_pool(name="vodps", bufs=1, space="PSUM") as vod_pool,
            tc.tile_pool(name="sps", bufs=1, space="PSUM") as s_pool,
            tc.tile_pool(name="qksb", bufs=2) as qksb_pool,
            tc.tile_pool(name="esb", bufs=2) as esb_pool,
            tc.tile_pool(name="drsb", bufs=2) as dr_pool,
            tc.tile_pool(name="hhsb", bufs=2) as hh_pool,
            tc.tile_pool(name="osb", bufs=2) as ost_pool,
            tc.tile_pool(name="xrsb", bufs=2) as xr_pool,
        ):
            pools = (qk_pool, vod_pool, s_pool, qksb_pool, esb_pool, dr_pool,
                     ost_pool, xr_pool)
            for sp in range(NSP):
                hh = hh_pool.tile([128, 512], BF16, tag="hh")
                super_stripe(sp, 0, yt, pools, hh[:], proj_dst=out_sp[sp])

    return nc


_NC_CACHE = {}


def _get_nc(has_qbias, has_pbias):
    key = (has_qbias, has_pbias)
    if key not in _NC_CACHE:
        nc = build_nc(has_qbias, has_pbias)
        nc.finalize()
        _NC_CACHE[key] = nc
    return _NC_CACHE[key]


def kernel(x, Wqkv, bqkv, Wproj, bproj, gamma, beta, _trace=False):
    x = np.asarray(x, np.float32)
    Wqkv = np.asarray(Wqkv, np.float32)
    bqkv = np.asarray(bqkv, np.float32)
    Wproj = np.asarray(Wproj, np.float32)
    bproj = np.asarray(bproj, np.float32)
    gamma = np.asarray(gamma, np.float32)
    beta = np.asarray(beta, np.float32)

    Wg = gamma[:, None] * Wqkv                      # fold LN affine scale
    bq = beta @ Wqkv + bqkv                         # fold LN affine shift
    has_qbias = bool(np.any(bq != 0.0))
    has_pbias = bool(np.any(bproj != 0.0))

    bf = ml_dtypes.bfloat16
    wqkv_np = np.ascontiguousarray(Wg.reshape(2, 128, 768)).astype(bf)
    wproj_np = np.ascontiguousarray(Wproj.reshape(2, 128, 256)).astype(bf)
    bq_np = bq.reshape(1, 768).astype(bf)
    bp_np = bproj.reshape(1, 256).astype(bf)
    eye_bf = np.eye(128, dtype=np.float32).astype(bf)
    eye_f32 = np.eye(128, dtype=np.float32)

    nc = _get_nc(has_qbias, has_pbias)
    in_maps = []
    for b in range(B):
        xb = np.ascontiguousarray(x[b].reshape(T, C))
        in_maps.append({
            "x": xb, "xr": xb,
            "wqkv": wqkv_np, "wproj": wproj_np,
            "bqkv": bq_np, "bproj": bp_np,
            "ident": eye_bf, "identr": eye_f32,
        })
    res = run_bass_kernel_spmd(nc, in_maps, list(range(B)), trace=_trace)
    out = np.stack([np.asarray(res.results[b]["out"]).reshape(HH, WW, C)
                    for b in range(B)])
    if _trace:
        return out.astype(np.float32), res
    return out.astype(np.float32)


# revision 16
# speedup vs baseline: 1.4584x; 1.4584x over previous
"""CSWin self-attention Trainium2 kernel.

Sharding: data-parallel over batch B=8 across 8 cores (1 image per core).
Per-core pipeline (image = 128x128 spatial, C=256):
  A) LayerNorm (gamma folded into Wqkv on host): bn_stats/aggr (DVE),
     batched rstd (one Sqrt + recip per 8 tiles), normalize on GPSIMD,
     PE-transpose, then two evictions of y^T: row-major yt (for the
     horizontal pass) and column-major ytT (for the vertical pass, so
     its qkv matmuls stream contiguously).
  B) Vertical stripes then horizontal stripes (+fused projection),
     processed as 32 "superstripes" of 2 stripes each:
       q^T/k^T matmuls at N=512 across both stripes, v token-major,
       one bf16 CAST for q/k + one for v,
       per stripe: S^T row-tiled 4 heads (K=32), exp on ScalarE
       ([128,2048] ACTIVATE, scale folded),
       attn@V col-tiled 4 heads + replicated denominators (ones lhsT)
       into a shared psum tile, batched reciprocal_approx_fast +
       tensor_mul straight from psum.
  C) (fused into horizontal pass) projection h @ Wproj + residual via
     float32r identity matmul on re-read x (full-speed at N=256);
     psum banks reused v->attn out->proj; output staged and DMA'd per
     superstripe.
"""

import math
from contextlib import ExitStack

import numpy as np
import ml_dtypes

import concourse.bass as bass
import concourse.bacc as bacc
import concourse.mybir as mybir
import concourse.tile as tile
from concourse.bass_utils import run_bass_kernel_spmd

F32 = mybir.dt.float32
F32R = mybir.dt.float32r
BF16 = mybir.dt.bfloat16
AF = mybir.ActivationFunctionType
ALU = mybir.AluOpType

B = 8
HH = 128
WW = 128
C = 256
T = HH * WW         # 16384 tokens
NT = T // 128       # 128 token tiles
NS = 64             # stripes per direction
NSP = NS // 2       # superstripes (2 stripes each)
SEQ = 256           # stripe seq len (2 * 128)
NHD = 4             # heads per direction
HD = 32
SCALE = HD ** -0.5
EPS = 1e-5
ABLK = 8            # tiles per rstd batch in phase A


def build_nc(has_qbias: bool, has_pbias: bool) -> bass.Bass:
    nc = bacc.Bacc("TRN2", target_bir_lowering=False, debug=False)
    x_h = nc.dram_tensor("x", [T, C], F32, kind="ExternalInput")
    xr_h = nc.dram_tensor("xr", [T, C], F32R, kind="ExternalInput")
    wqkv_h = nc.dram_tensor("wqkv", [2, 128, 768], BF16, kind="ExternalInput")
    wproj_h = nc.dram_tensor("wproj", [2, 128, 256], BF16, kind="ExternalInput")
    bqkv_h = nc.dram_tensor("bqkv", [1, 768], BF16, kind="ExternalInput")
    bproj_h = nc.dram_tensor("bproj", [1, 256], BF16, kind="ExternalInput")
    ident_h = nc.dram_tensor("ident", [128, 128], BF16, kind="ExternalInput")
    identr_h = nc.dram_tensor("identr", [128, 128], F32R, kind="ExternalInput")
    out_h = nc.dram_tensor("out", [T, C], F32, kind="ExternalOutput")

    with tile.TileContext(nc) as tc, tc.tile_pool(name="persist", bufs=1) as pp:
        # ---------------- persistent SBUF ----------------
        yt = pp.tile([128, 2, T], BF16, name="yt", tag="yt")
        ytT = pp.tile([128, 2, T], BF16, name="ytT", tag="ytT")
        hVt = pp.tile([128, T], BF16, name="hVt", tag="hVt")
        wqkv = pp.tile([128, 2 * 768], BF16, name="wqkv", tag="wqkv")
        wproj = pp.tile([128, 2 * 256], BF16, name="wproj", tag="wproj")
        brow = pp.tile([1, 768], BF16, name="brow", tag="brow")
        bprow = pp.tile([1, 256], BF16, name="bprow", tag="bprow")
        ones = pp.tile([1, 512], BF16, name="ones", tag="ones")
        ones32 = pp.tile([128, 32], BF16, name="ones32", tag="ones32")
        ident = pp.tile([128, 128], BF16, name="ident", tag="ident")
        identr = pp.tile([128, 128], F32R, name="identr", tag="identr")
        mvs = pp.tile([128, NT, 2], F32, name="mvs", tag="mvs")
        srt = pp.tile([128, NT], F32, name="srt", tag="srt")
        rstds = pp.tile([128, NT], F32, name="rstds", tag="rstds")
        negmr = pp.tile([128, NT], F32, name="negmr", tag="negmr")
        epsc = pp.tile([128, 1], F32, name="epsc", tag="epsc")

        nc.sync.dma_start(out=wqkv[:, 0:768], in_=wqkv_h[0])
        nc.sync.dma_start(out=wqkv[:, 768:1536], in_=wqkv_h[1])
        nc.sync.dma_start(out=wproj[:, 0:256], in_=wproj_h[0])
        nc.sync.dma_start(out=wproj[:, 256:512], in_=wproj_h[1])
        if has_qbias:
            nc.sync.dma_start(out=brow[:], in_=bqkv_h[:])
        if has_pbias:
            nc.sync.dma_start(out=bprow[:], in_=bproj_h[:])
        nc.vector.memset(ones[:], 1.0)
        nc.vector.memset(ones32[:], 1.0)
        nc.vector.memset(epsc[:], EPS)
        nc.sync.dma_start(out=ident[:], in_=ident_h[:, :])
        nc.sync.dma_start(out=identr[:], in_=identr_h[:, :])

        ytTv = ytT[:].rearrange("p c (w h) -> p c w h", w=WW)
        x_blk = x_h[:, :].rearrange("(b t p) c -> b p t c", t=ABLK, p=128)

        # ---------------- phase A: LN + transpose ----------------
        with (
            tc.tile_pool(name="xa", bufs=2) as xa_pool,
            tc.tile_pool(name="ya", bufs=3) as ya_pool,
            tc.tile_pool(name="sa", bufs=4) as sa_pool,
            tc.tile_pool(name="tp", bufs=3, space="PSUM") as tp_pool,
        ):
            for blk in range(NT // ABLK):
                xs = xa_pool.tile([128, ABLK, 256], F32, tag="xs")
                nc.sync.dma_start(out=xs[:], in_=x_blk[blk])
                for j in range(ABLK):
                    i = blk * ABLK + j
                    st6 = sa_pool.tile([128, 6], F32, tag="st6")
                    nc.vector.bn_stats(st6[:], xs[:, j, :])
                    nc.vector.bn_aggr(mvs[:, i, :], st6[:])
                sl = slice(blk * ABLK, (blk + 1) * ABLK)
                nc.scalar.activation(srt[:, sl], mvs[:, sl, 1], AF.Sqrt,
                                     bias=epsc[:])
                nc.vector.reciprocal(rstds[:, sl], srt[:, sl])
                nc.vector.scalar_tensor_tensor(
                    negmr[:, sl], mvs[:, sl, 0], -1.0, rstds[:, sl],
                    ALU.mult, ALU.mult)
                for j in range(ABLK):
                    i = blk * ABLK + j
                    yn = ya_pool.tile([128, 256], BF16, tag="yn")
                    if j % 2 == 0:
                        nc.vector.tensor_scalar(
                            yn[:], xs[:, j, :], mvs[:, i, 0:1],
                            rstds[:, i:i + 1], ALU.subtract, ALU.mult,
                        )
                    else:
                        nc.scalar.activation(
                            yn[:], xs[:, j, :], AF.Identity,
                            bias=negmr[:, i:i + 1], scale=rstds[:, i:i + 1],
                        )
                    tp = tp_pool.tile([128, 2, 128], BF16, tag="tp")
                    nc.tensor.transpose(tp[:, 0, :], yn[:, 0:128], ident[:])
                    nc.tensor.transpose(tp[:, 1, :], yn[:, 128:256], ident[:])
                    nc.scalar.copy(yt[:, :, i * 128:(i + 1) * 128], tp[:])
                    nc.scalar.copy(ytTv[:, :, :, i], tp[:])

        # stripe-sliced channel-major views of y^T
        hVv = hVt[:].rearrange("p (g j h) -> p g j h", g=NS, j=2)
        out_sp = out_h[:, :].rearrange("(t j p) c -> t p j c", j=4, p=128)
        xr_sp = xr_h[:, :].rearrange("(t j p) c -> t p j c", j=4, p=128)

        def super_stripe(sp, qoff, ysrc, pools, mdst, proj_dst=None):
            """Attention for superstripe sp (stripes 2sp, 2sp+1).
            mdst: [128, 512] AP for normalized h^T of both stripes.
            proj_dst: out_sp[sp]-style AP to enable fused projection."""
            (qk_pool, vod_pool, s_pool, qksb_pool, esb_pool, dr_pool,
             ost_pool, xr_pool) = pools
            tok0 = sp * 512
            yv = ysrc[:]  # [128, 2, T]
            qk_ps = qk_pool.tile([128, 1024], F32, tag="qkps")
            vod = vod_pool.tile([128, 1024], F32, tag="vod")
            # emission order separates same-region accumulation pairs so
            # consecutive matmuls hitting one psum region don't stall the PE
            for kc in range(2):
                wof = kc * 768
                rv = yv[:, kc, tok0:tok0 + 512]
                nc.tensor.matmul(
                    qk_ps[:, 0:512], lhsT=wqkv[:, wof + qoff:wof + qoff + 128],
                    rhs=rv, start=kc == 0, stop=kc == 1 and not has_qbias)
                nc.tensor.matmul(
                    qk_ps[:, 512:1024],
                    lhsT=wqkv[:, wof + 256 + qoff:wof + 384 + qoff],
                    rhs=rv, start=kc == 0, stop=kc == 1 and not has_qbias)
                if kc == 0:
                    for kc2 in range(2):
                        wof2 = kc2 * 768
                        for q in range(4):  # (stripe s, seq half sc) quarters
                            nc.tensor.matmul(
                                vod[:, q * 128:q * 128 + 128],
                                lhsT=yv[:, kc2,
                                        tok0 + q * 128:tok0 + (q + 1) * 128],
                                rhs=wqkv[:, wof2 + 512 + qoff:
                                         wof2 + 640 + qoff],
                                start=kc2 == 0,
                                stop=kc2 == 1 and not has_qbias)
            if has_qbias:
                nc.tensor.matmul(
                    qk_ps[:, 0:512], lhsT=brow[:, qoff:qoff + 128],
                    rhs=ones[:, 0:512], start=False, stop=True)
                nc.tensor.matmul(
                    qk_ps[:, 512:1024], lhsT=brow[:, 256 + qoff:384 + qoff],
                    rhs=ones[:, 0:512], start=False, stop=True)
                for q in range(4):
                    nc.tensor.matmul(
                        vod[:, q * 128:q * 128 + 128],
                        lhsT=ones[:, 0:128],
                        rhs=brow[:, 512 + qoff:640 + qoff],
                        start=False, stop=True)
            qkv_sb = qksb_pool.tile([128, 1536], BF16, tag="qkvsb")
            nc.vector.tensor_copy(qkv_sb[:, 0:1024], qk_ps[:])
            nc.vector.tensor_copy(qkv_sb[:, 1024:1536], vod[:, 0:512])
            # per (stripe, head-pair): S^T, exp; then attn@V + denominators.
            # s_ps holds 2 heads ([128,1024], 2 psum banks) so two tiles
            # pipeline: exp of one half overlaps S matmuls of the next.
            e_sbs = [[None, None], [None, None]]
            for s in range(2):
                for hp in range(2):  # head pair: heads 2hp, 2hp+1
                    s_ps = s_pool.tile([128, 1024], F32, tag="sps")
                    for h2 in range(2):
                        h = 2 * hp + h2
                        for sc in range(2):
                            nc.tensor.matmul(
                                s_ps[:, h2 * 512 + sc * 256:
                                     h2 * 512 + sc * 256 + 256],
                                lhsT=qkv_sb[32 * h:32 * h + 32,
                                            512 + s * 256 + sc * 128:
                                            512 + s * 256 + sc * 128 + 128],
                                rhs=qkv_sb[32 * h:32 * h + 32,
                                           s * 256:s * 256 + 256],
                                start=True, stop=True,
                                tile_position=(32 * h, 0))
                    e_sb = esb_pool.tile([128, 1024], BF16, tag="esb")
                    nc.scalar.activation(e_sb[:], s_ps[:], AF.Exp, scale=SCALE)
                    e_sbs[s][hp] = e_sb
            for s in range(2):
                for sc in range(2):
                    for h in range(NHD):
                        e_sb = e_sbs[s][h // 2]
                        eo = (h % 2) * 512 + sc * 256
                        nc.tensor.matmul(
                            vod[32 * h:32 * h + 32, s * 256:s * 256 + 256],
                            lhsT=qkv_sb[:, 1024 + s * 256 + sc * 128 + 32 * h:
                                        1024 + s * 256 + sc * 128 + 32 * h + 32],
                            rhs=e_sb[:, eo:eo + 256],
                            start=sc == 0, stop=sc == 1,
                            tile_position=(0, 32 * h))
                    for h in range(NHD):
                        e_sb = e_sbs[s][h // 2]
                        eo = (h % 2) * 512 + sc * 256
                        nc.tensor.matmul(
                            vod[32 * h:32 * h + 32,
                                512 + s * 256:512 + s * 256 + 256],
                            lhsT=ones32[:],
                            rhs=e_sb[:, eo:eo + 256],
                            start=sc == 0, stop=sc == 1,
                            tile_position=(0, 32 * h))
            drec = dr_pool.tile([128, 512], F32, tag="drec")
            nc.vector.reciprocal_approx_fast(drec[:], vod[:, 512:1024])
            nc.vector.tensor_mul(mdst, vod[:, 0:512], drec[:])
            if proj_dst is None:
                return
            # ---- fused projection + residual for 4 token tiles ----
            xrs = xr_pool.tile([128, 4, 256], F32R, tag="xrs")
            nc.sync.dma_start(out=xrs[:], in_=xr_sp[sp])
            for jj in range(4):
                t = 4 * sp + jj
                po = slice(jj * 256, jj * 256 + 256)
                nc.tensor.matmul(
                    vod[:, po], lhsT=mdst[:, jj * 128:jj * 128 + 128],
                    rhs=wproj[:, 0:256], start=True, stop=False)
                nc.tensor.matmul(
                    vod[:, po], lhsT=hVv[:, :, :, t],
                    rhs=wproj[:, 256:512], start=False, stop=False)
                nc.tensor.matmul(
                    vod[:, po], lhsT=identr[:],
                    rhs=xrs[:, jj, :], start=False, stop=not has_pbias)
                if has_pbias:
                    nc.tensor.matmul(
                        vod[:, po], lhsT=ones[:, 0:128], rhs=bprow[:],
                        start=False, stop=True)
            ost = ost_pool.tile([128, 4, 256], F32, tag="ost")
            nc.scalar.copy(ost[:, 0:2, :], vod[:, 0:512])
            nc.vector.tensor_copy(ost[:, 2:4, :], vod[:, 512:1024])
            nc.sync.dma_start(out=proj_dst, in_=ost[:])

        # ---------------- phase Bv: vertical attention ----------------
        with (
            tc.tile_pool(name="qkps", bufs=1, space="PSUM") as qk_pool,
            tc.tile_pool(name="vodps", bufs=1, space="PSUM") as vod_pool,
            tc.tile_pool(name="sps", bufs=2, space="PSUM") as s_pool,
            tc.tile_pool(name="qksb", bufs=2) as qksb_pool,
            tc.tile_pool(name="esb", bufs=4) as esb_pool,
            tc.tile_pool(name="drsb", bufs=2) as dr_pool,
        ):
            pools = (qk_pool, vod_pool, s_pool, qksb_pool, esb_pool, dr_pool,
                     None, None)
            for sp in range(NSP):
                super_stripe(sp, 128, ytT, pools,
                             hVt[:, sp * 512:(sp + 1) * 512])

        # ------------- phase BhC: horizontal attention + proj -------------
        with (
            tc.tile_pool(name="qkps", bufs=1, space="PSUM") as qk_pool,
            tc.tile_pool(name="vodps", bufs=1, space="PSUM") as vod_pool,
            tc.tile_pool(name="sps", bufs=2, space="PSUM") as s_pool,
            tc.tile_pool(name="qksb", bufs=2) as qksb_pool,
            tc.tile_pool(name="esb", bufs=4) as esb_pool,
            tc.tile_pool(name="drsb", bufs=2) as dr_pool,
            tc.tile_pool(name="hhsb", bufs=2) as hh_pool,
            tc.tile_pool(name="osb", bufs=2) as ost_pool,
            tc.tile_pool(name="xrsb", bufs=2) as xr_pool,
        ):
            pools = (qk_pool, vod_pool, s_pool, qksb_pool, esb_pool, dr_pool,
                     ost_pool, xr_pool)
            for sp in range(NSP):
                hh = hh_pool.tile([128, 512], BF16, tag="hh")
                super_stripe(sp, 0, yt, pools, hh[:], proj_dst=out_sp[sp])

    return nc


_NC_CACHE = {}


def _get_nc(has_qbias, has_pbias):
    key = (has_qbias, has_pbias)
    if key not in _NC_CACHE:
        nc = build_nc(has_qbias, has_pbias)
        nc.finalize()
        _NC_CACHE[key] = nc
    return _NC_CACHE[key]


def kernel(x, Wqkv, bqkv, Wproj, bproj, gamma, beta, _trace=False):
    x = np.asarray(x, np.float32)
    Wqkv = np.asarray(Wqkv, np.float32)
    bqkv = np.asarray(bqkv, np.float32)
    Wproj = np.asarray(Wproj, np.float32)
    bproj = np.asarray(bproj, np.float32)
    gamma = np.asarray(gamma, np.float32)
    beta = np.asarray(beta, np.float32)

    Wg = gamma[:, None] * Wqkv                      # fold LN affine scale
    bq = beta @ Wqkv + bqkv                         # fold LN affine shift
    has_qbias = bool(np.any(bq != 0.0))
    has_pbias = bool(np.any(bproj != 0.0))

    bf = ml_dtypes.bfloat16
    wqkv_np = np.ascontiguousarray(Wg.reshape(2, 128, 768)).astype(bf)
    wproj_np = np.ascontiguousarray(Wproj.reshape(2, 128, 256)).astype(bf)
    bq_np = bq.reshape(1, 768).astype(bf)
    bp_np = bproj.reshape(1, 256).astype(bf)
    eye_bf = np.eye(128, dtype=np.float32).astype(bf)
    eye_f32 = np.eye(128, dtype=np.float32)

    nc = _get_nc(has_qbias, has_pbias)
    in_maps = []
    for b in range(B):
        xb = np.ascontiguousarray(x[b].reshape(T, C))
        in_maps.append({
            "x": xb, "xr": xb,
            "wqkv": wqkv_np, "wproj": wproj_np,
            "bqkv": bq_np, "bproj": bp_np,
            "ident": eye_bf, "identr": eye_f32,
        })
    res = run_bass_kernel_spmd(nc, in_maps, list(range(B)), trace=_trace)
    out = np.stack([np.asarray(res.results[b]["out"]).reshape(HH, WW, C)
                    for b in range(B)])
    if _trace:
        return out.astype(np.float32), res
    return out.astype(np.float32)
